# revision 5
# baseline (speedup 1.0000x reference)
"""LogSumExpWirelength on 8 TRN2 NeuronCores — fully on-device version.

Launch 1 (pins sharded 8x): per chunk, ACT computes exp(+-x/g), exp(+-y/g);
then one indirect-DMA scatter-ADD per 128-pin column accumulates the 4 exp
components into per-net DRAM tables. 8 rotating table lanes make consecutive
calls target disjoint DRAM tiles, so Tile's WAW serialization per lane never
stalls the pipeline and concurrent CCE read-modify-writes to the same net
cannot race (a same-net collision inside one 128-pin call is still summed
correctly by CCE add order on one engine queue; across calls the lane
rotation + per-lane ordering protects the RMW window).  Lanes are then
dense-merged on device into one [NETS_PAD, 4] table per core.

Launch 2 (nets sharded 8x): each core receives its slice of all 8 cores'
tables, tree-sums them, applies log, the S>0 empty-net guard and net_mask,
and reduces to [128,1] partials.  Host work is only slicing and a final
1024-element sum (+ gamma scale).
"""

import time

import numpy as np

import concourse.bass as bass
import concourse.mybir as mybir
import concourse.tile as tile
from concourse.bass_utils import run_bass_kernel_spmd

NUM_PINS = 16777216
NUM_NETS = 4000000
GAMMA = 0.5
N_CORES = 8

NETS_PAD = 1 << 22                       # 4194304
PINS_PER_CORE = NUM_PINS // N_CORES      # 2097152
NETS_PER_CORE = NETS_PAD // N_CORES      # 524288 = 128 * 4096

_SC_COLS = 2048                          # pins per partition per chunk
_SC_CHUNK = 128 * _SC_COLS
_LANES = 8
_PB_COLS = 4096

# ---------------------------------------------------------------------------
# Workarounds for this container's walrus build: it allows at most ONE
# sync-wait command per instruction.  Tile's tail drain and its scheduler
# both attach several; split the excess onto same-engine Drain carriers.
# ---------------------------------------------------------------------------
_MAX_WAITS = 1


def _patched_drain_and_barrier(self, tick_clock, wait_clock):
    from concourse.tile import ScopedClock

    drain_inst = self.nc.sync.drain()
    wait_clock.add_sem_waits(
        drain_inst.ins, ScopedClock({None: tick_clock.global_clock})
    )
    mi = drain_inst.ins
    waits = list(mi.sync_info.on_wait)
    if len(waits) > _MAX_WAITS:
        si = mi.sync_info
        si.on_wait = waits[:_MAX_WAITS]
        mi.sync_info = si
        rest = waits[_MAX_WAITS:]
        while rest:
            d = self.nc.sync.drain()
            d.ins.sync_info = mybir.SyncInfo(
                on_wait=rest[:_MAX_WAITS], on_update=[]
            )
            rest = rest[_MAX_WAITS:]
    self.nc.all_engine_barrier()
    popped = self.nc._tile_sem_poison_stack.pop()
    assert popped is self._sem_poison
    self.nc.clear_and_free_semaphores(list(self.sems.allocated().values()))
    self.nc.all_engine_barrier()


tile.TileContext._drain_and_barrier = _patched_drain_and_barrier


def _split_waits(nc):
    """Move excess sync-waits onto same-engine Drain carriers in front."""
    k = 0
    for f in nc.m.functions:
        for bb in f.blocks:
            insts = list(bb.instructions)
            out = []
            changed = False
            for inst in insts:
                si = inst.sync_info
                if si is not None and len(si.on_wait) > _MAX_WAITS:
                    waits = list(si.on_wait)
                    for w in waits[:-_MAX_WAITS]:
                        k += 1
                        d = mybir.InstDrain(name=f"WS-{k}", ins=[], outs=[])
                        d.engine = inst.engine
                        d.sync_info = mybir.SyncInfo(on_wait=[w], on_update=[])
                        out.append(d)
                    si.on_wait = waits[-_MAX_WAITS:]
                    inst.sync_info = si
                    changed = True
                out.append(inst)
            if changed:
                bb.instructions = out



_nc_cache = {}
LAUNCH_WALLS = {}


def _build_scatter():
    nc = bass.Bass("TRN2", target_bir_lowering=False, debug=False,
                   num_devices=N_CORES)
    x_in = nc.dram_tensor("x", [PINS_PER_CORE], mybir.dt.float32,
                          kind="ExternalInput")
    y_in = nc.dram_tensor("y", [PINS_PER_CORE], mybir.dt.float32,
                          kind="ExternalInput")
    n_in = nc.dram_tensor("net", [PINS_PER_CORE], mybir.dt.int32,
                          kind="ExternalInput")
    tab_out = nc.dram_tensor("tab", [NETS_PAD, 4], mybir.dt.bfloat16,
                             kind="ExternalOutput")
    inv_g = 1.0 / GAMMA
    n_chunks = PINS_PER_CORE // _SC_CHUNK
    with tile.TileContext(nc) as tc:
        with tc.tile_pool(name="sb", bufs=2) as pool, \
             tc.tile_pool(name="zb", bufs=1) as zpool, \
             tc.tile_pool(name="dram", bufs=1, space="DRAM") as dpool:
            tables = []
            for l in range(_LANES):
                tables.append(
                    dpool.tile([NETS_PAD, 4], mybir.dt.bfloat16,
                               name=f"lane{l}", tag=f"lane{l}")
                )
            # zero all lanes: 128 partitions x 8192 f32 = 4MiB per DMA
            zt = zpool.tile([128, 8192], mybir.dt.bfloat16)
            nc.vector.memset(zt[:], 0.0)
            for l in range(_LANES):
                v = tables[l][:].rearrange(
                    "(a p f) d -> a p (f d)", p=128, f=2048)
                for a in range(NETS_PAD * 4 // (128 * 8192)):
                    nc.sync.dma_start(out=v[a], in_=zt[:])
            bc_reg = nc.gpsimd.to_reg(NETS_PAD - 1)
            call = 0
            for c in range(n_chunks):
                sl = slice(c * _SC_CHUNK, (c + 1) * _SC_CHUNK)
                nt = pool.tile([128, _SC_COLS], mybir.dt.int32, tag="nt")
                nc.sync.dma_start(
                    out=nt[:], in_=n_in[sl].rearrange("(p t) -> p t", p=128))
                v4f = pool.tile([128, _SC_COLS, 4], mybir.dt.float32,
                                tag="v4f")
                v4 = pool.tile([128, _SC_COLS, 4], mybir.dt.bfloat16,
                               tag="v4")
                for src, outs_k in ((x_in, (0, 1)), (y_in, (2, 3))):
                    t = pool.tile([128, _SC_COLS], mybir.dt.float32, tag="xy")
                    nc.sync.dma_start(
                        out=t[:], in_=src[sl].rearrange("(p t) -> p t", p=128))
                    for k, s in zip(outs_k, (inv_g, -inv_g)):
                        nc.scalar.activation(
                            v4f[:, :, k], t[:],
                            mybir.ActivationFunctionType.Exp, scale=s)
                nc.vector.tensor_copy(v4[:], v4f[:])
                for col in range(_SC_COLS):
                    nc.gpsimd.indirect_dma_start(
                        out=tables[call % _LANES][:],
                        out_offset=bass.IndirectOffsetOnAxis(
                            ap=nt[:, col:col + 1], axis=0),
                        in_=v4[:, col, :],
                        in_offset=None,
                        bounds_check=bc_reg,
                        oob_is_err=False,
                        compute_op=mybir.AluOpType.add,
                    )
                    call += 1
            # dense-merge lanes into tab_out
            n_m = NETS_PAD * 4 // (128 * 2048)
            for a in range(n_m):
                acc = pool.tile([128, 2048], mybir.dt.bfloat16, tag="macc")
                nc.sync.dma_start(
                    out=acc[:],
                    in_=tables[0][:].rearrange(
                        "(a p f) d -> a p (f d)", p=128, f=512)[a])
                for l in range(1, _LANES):
                    tl = pool.tile([128, 2048], mybir.dt.bfloat16, tag="mtl")
                    nc.sync.dma_start(
                        out=tl[:],
                        in_=tables[l][:].rearrange(
                            "(a p f) d -> a p (f d)", p=128, f=512)[a])
                    nc.vector.tensor_tensor(
                        out=acc[:], in0=acc[:], in1=tl[:],
                        op=mybir.AluOpType.add)
                nc.sync.dma_start(
                    out=tab_out[:].rearrange(
                        "(a p f) d -> a p (f d)", p=128, f=512)[a],
                    in_=acc[:])
    _split_waits(nc)
    return nc


def _build_reduce():
    nc = bass.Bass("TRN2", target_bir_lowering=False, debug=False,
                   num_devices=N_CORES)
    t_in = [
        nc.dram_tensor(f"t{j}", [NETS_PER_CORE, 4], mybir.dt.bfloat16,
                       kind="ExternalInput")
        for j in range(N_CORES)
    ]
    m_in = nc.dram_tensor("mask", [NETS_PER_CORE], mybir.dt.uint8,
                          kind="ExternalInput")
    p_out = nc.dram_tensor("partial", [128, 1], mybir.dt.float32,
                           kind="ExternalOutput")
    NB = 4
    FB = _PB_COLS // NB          # nets per partition per block
    with tile.TileContext(nc) as tc:
        with tc.tile_pool(name="sb", bufs=2) as pool, \
             tc.tile_pool(name="ac", bufs=1) as apool:
            tot = apool.tile([128, 1], mybir.dt.float32)
            nc.vector.memset(tot[:], 0.0)
            for b in range(NB):
                s = pool.tile([128, FB * 4], mybir.dt.float32, tag="s")
                view = lambda j: t_in[j][:].rearrange(
                    "(p nb f) d -> p nb (f d)", p=128, nb=NB)[:, b]
                s0 = pool.tile([128, FB * 4], mybir.dt.bfloat16, tag="s0")
                nc.sync.dma_start(out=s0[:], in_=view(0))
                nc.vector.tensor_copy(s[:], s0[:])
                for j in range(1, N_CORES):
                    tj = pool.tile([128, FB * 4], mybir.dt.bfloat16, tag="tj")
                    nc.sync.dma_start(out=tj[:], in_=view(j))
                    nc.vector.tensor_tensor(
                        out=s[:], in0=s[:], in1=tj[:], op=mybir.AluOpType.add)
                pos = pool.tile([128, FB * 4], mybir.dt.float32, tag="pos")
                nc.vector.tensor_scalar(
                    pos[:], s[:], 0.0, None, op0=mybir.AluOpType.is_gt)
                nc.vector.tensor_scalar_add(s[:], s[:], 1e-30)
                ln = pool.tile([128, FB * 4], mybir.dt.float32, tag="ln")
                nc.scalar.activation(
                    ln[:], s[:], mybir.ActivationFunctionType.Ln)
                nc.vector.tensor_tensor(
                    out=ln[:], in0=ln[:], in1=pos[:], op=mybir.AluOpType.mult)
                wl = pool.tile([128, FB], mybir.dt.float32, tag="wl")
                nc.vector.tensor_reduce(
                    out=wl[:], in_=ln[:].rearrange("p (f d) -> p f d", d=4),
                    axis=mybir.AxisListType.X, op=mybir.AluOpType.add)
                mu8 = pool.tile([128, FB], mybir.dt.uint8, tag="mu8")
                nc.sync.dma_start(
                    out=mu8[:],
                    in_=m_in[:].rearrange("(p nb f) -> p nb f", p=128, nb=NB)[:, b])
                mf = pool.tile([128, FB], mybir.dt.float32, tag="mf")
                nc.vector.tensor_scalar(
                    mf[:], mu8[:], 0, None, op0=mybir.AluOpType.is_gt)
                nc.vector.tensor_tensor(
                    out=wl[:], in0=wl[:], in1=mf[:], op=mybir.AluOpType.mult)
                red = pool.tile([128, 1], mybir.dt.float32, tag="red")
                nc.vector.tensor_reduce(
                    out=red[:], in_=wl[:], axis=mybir.AxisListType.X,
                    op=mybir.AluOpType.add)
                nc.vector.tensor_tensor(
                    out=tot[:], in0=tot[:], in1=red[:], op=mybir.AluOpType.add)
            nc.sync.dma_start(out=p_out[:], in_=tot[:])
    _split_waits(nc)
    return nc


def _get(name, builder):
    if name not in _nc_cache:
        _nc_cache[name] = builder()
    return _nc_cache[name]


def kernel(pos, pin2net_map, net_mask):
    pos = np.asarray(pos, dtype=np.float32)
    pin2net_map = np.asarray(pin2net_map, dtype=np.int32)
    net_mask = np.asarray(net_mask)

    x = pos[:NUM_PINS]
    y = pos[NUM_PINS:]

    nc_s = _get("s", _build_scatter)
    in_maps = []
    for i in range(N_CORES):
        sl = slice(i * PINS_PER_CORE, (i + 1) * PINS_PER_CORE)
        in_maps.append({
            "x": np.ascontiguousarray(x[sl]),
            "y": np.ascontiguousarray(y[sl]),
            "net": np.ascontiguousarray(pin2net_map[sl]),
        })
    t0 = time.time()
    res_s = run_bass_kernel_spmd(nc_s, in_maps, list(range(N_CORES)))
    LAUNCH_WALLS["scatter"] = time.time() - t0
    tabs = [res_s.results[i]["tab"] for i in range(N_CORES)]

    maskp = np.zeros(NETS_PAD, dtype=np.uint8)
    maskp[:NUM_NETS] = net_mask.astype(np.uint8)

    nc_r = _get("r", _build_reduce)
    in_maps_r = []
    for i in range(N_CORES):
        sl = slice(i * NETS_PER_CORE, (i + 1) * NETS_PER_CORE)
        m = {f"t{j}": np.ascontiguousarray(tabs[j][sl]) for j in range(N_CORES)}
        m["mask"] = np.ascontiguousarray(maskp[sl])
        in_maps_r.append(m)
    t0 = time.time()
    res_r = run_bass_kernel_spmd(nc_r, in_maps_r, list(range(N_CORES)))
    LAUNCH_WALLS["reduce"] = time.time() - t0
    total = 0.0
    for i in range(N_CORES):
        total += float(res_r.results[i]["partial"].sum())
    return np.float32(GAMMA * total)


# revision 6
# speedup vs baseline: 1.5158x; 1.5158x over previous
"""LogSumExpWirelength on 8 TRN2 NeuronCores — fully on-device version.

Launch 1 (pins sharded 8x): per chunk, ACT computes exp(+-x/g), exp(+-y/g);
then one indirect-DMA scatter-ADD per 128-pin column accumulates the 4 exp
components into per-net DRAM tables. 8 rotating table lanes make consecutive
calls target disjoint DRAM tiles, so Tile's WAW serialization per lane never
stalls the pipeline and concurrent CCE read-modify-writes to the same net
cannot race (a same-net collision inside one 128-pin call is still summed
correctly by CCE add order on one engine queue; across calls the lane
rotation + per-lane ordering protects the RMW window).  Lanes are then
dense-merged on device into one [NETS_PAD, 4] table per core.

Launch 2 (nets sharded 8x): each core receives its slice of all 8 cores'
tables, tree-sums them, applies log, the S>0 empty-net guard and net_mask,
and reduces to [128,1] partials.  Host work is only slicing and a final
1024-element sum (+ gamma scale).
"""

import time

import numpy as np

import concourse.bass as bass
import concourse.mybir as mybir
import concourse.tile as tile
from concourse.bass_utils import run_bass_kernel_spmd

NUM_PINS = 16777216
NUM_NETS = 4000000
GAMMA = 0.5
N_CORES = 8

NETS_PAD = 1 << 22                       # 4194304
PINS_PER_CORE = NUM_PINS // N_CORES      # 2097152
NETS_PER_CORE = NETS_PAD // N_CORES      # 524288 = 128 * 4096

_SC_COLS = 2048                          # pins per partition per chunk
_SC_CHUNK = 128 * _SC_COLS
_LANES = 8
_PB_COLS = 4096

# ---------------------------------------------------------------------------
# Workarounds for this container's walrus build: it allows at most ONE
# sync-wait command per instruction.  Tile's tail drain and its scheduler
# both attach several; split the excess onto same-engine Drain carriers.
# ---------------------------------------------------------------------------
_MAX_WAITS = 1


def _patched_drain_and_barrier(self, tick_clock, wait_clock):
    from concourse.tile import ScopedClock

    drain_inst = self.nc.sync.drain()
    wait_clock.add_sem_waits(
        drain_inst.ins, ScopedClock({None: tick_clock.global_clock})
    )
    mi = drain_inst.ins
    waits = list(mi.sync_info.on_wait)
    if len(waits) > _MAX_WAITS:
        si = mi.sync_info
        si.on_wait = waits[:_MAX_WAITS]
        mi.sync_info = si
        rest = waits[_MAX_WAITS:]
        while rest:
            d = self.nc.sync.drain()
            d.ins.sync_info = mybir.SyncInfo(
                on_wait=rest[:_MAX_WAITS], on_update=[]
            )
            rest = rest[_MAX_WAITS:]
    self.nc.all_engine_barrier()
    popped = self.nc._tile_sem_poison_stack.pop()
    assert popped is self._sem_poison
    self.nc.clear_and_free_semaphores(list(self.sems.allocated().values()))
    self.nc.all_engine_barrier()


tile.TileContext._drain_and_barrier = _patched_drain_and_barrier


def _split_waits(nc):
    """Move excess sync-waits onto same-engine Drain carriers in front."""
    k = 0
    for f in nc.m.functions:
        for bb in f.blocks:
            insts = list(bb.instructions)
            out = []
            changed = False
            for inst in insts:
                si = inst.sync_info
                if si is not None and len(si.on_wait) > _MAX_WAITS:
                    waits = list(si.on_wait)
                    for w in waits[:-_MAX_WAITS]:
                        k += 1
                        d = mybir.InstDrain(name=f"WS-{k}", ins=[], outs=[])
                        d.engine = inst.engine
                        d.sync_info = mybir.SyncInfo(on_wait=[w], on_update=[])
                        out.append(d)
                    si.on_wait = waits[-_MAX_WAITS:]
                    inst.sync_info = si
                    changed = True
                out.append(inst)
            if changed:
                bb.instructions = out



_nc_cache = {}
LAUNCH_WALLS = {}


def _build_scatter():
    nc = bass.Bass("TRN2", target_bir_lowering=False, debug=False,
                   num_devices=N_CORES)
    x_in = nc.dram_tensor("x", [PINS_PER_CORE], mybir.dt.float16,
                          kind="ExternalInput")
    y_in = nc.dram_tensor("y", [PINS_PER_CORE], mybir.dt.float16,
                          kind="ExternalInput")
    n_in = nc.dram_tensor("net", [PINS_PER_CORE], mybir.dt.int32,
                          kind="ExternalInput")
    tab_out = nc.dram_tensor("tab", [NETS_PAD, 4], mybir.dt.bfloat16,
                             kind="ExternalOutput")
    inv_g = 1.0 / GAMMA
    n_chunks = PINS_PER_CORE // _SC_CHUNK
    with tile.TileContext(nc) as tc:
        with tc.tile_pool(name="sb", bufs=2) as pool, \
             tc.tile_pool(name="zb", bufs=1) as zpool, \
             tc.tile_pool(name="dram", bufs=1, space="DRAM") as dpool:
            tables = []
            for l in range(_LANES):
                tables.append(
                    dpool.tile([NETS_PAD, 4], mybir.dt.bfloat16,
                               name=f"lane{l}", tag=f"lane{l}")
                )
            # zero all lanes: 128 partitions x 8192 f32 = 4MiB per DMA
            zt = zpool.tile([128, 8192], mybir.dt.bfloat16)
            nc.vector.memset(zt[:], 0.0)
            for l in range(_LANES):
                v = tables[l][:].rearrange(
                    "(a p f) d -> a p (f d)", p=128, f=2048)
                for a in range(NETS_PAD * 4 // (128 * 8192)):
                    nc.sync.dma_start(out=v[a], in_=zt[:])
            bc_reg = nc.gpsimd.to_reg(NETS_PAD - 1)
            call = 0
            for c in range(n_chunks):
                sl = slice(c * _SC_CHUNK, (c + 1) * _SC_CHUNK)
                nt = pool.tile([128, _SC_COLS], mybir.dt.int32, tag="nt")
                nc.sync.dma_start(
                    out=nt[:], in_=n_in[sl].rearrange("(p t) -> p t", p=128))
                v4f = pool.tile([128, _SC_COLS, 4], mybir.dt.float32,
                                tag="v4f")
                v4 = pool.tile([128, _SC_COLS, 4], mybir.dt.bfloat16,
                               tag="v4")
                for src, outs_k in ((x_in, (0, 1)), (y_in, (2, 3))):
                    t = pool.tile([128, _SC_COLS], mybir.dt.float16, tag="xy")
                    nc.sync.dma_start(
                        out=t[:], in_=src[sl].rearrange("(p t) -> p t", p=128))
                    for k, s in zip(outs_k, (inv_g, -inv_g)):
                        nc.scalar.activation(
                            v4f[:, :, k], t[:],
                            mybir.ActivationFunctionType.Exp, scale=s)
                nc.vector.tensor_copy(v4[:], v4f[:])
                for col in range(_SC_COLS):
                    nc.gpsimd.indirect_dma_start(
                        out=tables[call % _LANES][:],
                        out_offset=bass.IndirectOffsetOnAxis(
                            ap=nt[:, col:col + 1], axis=0),
                        in_=v4[:, col, :],
                        in_offset=None,
                        bounds_check=bc_reg,
                        oob_is_err=False,
                        compute_op=mybir.AluOpType.add,
                    )
                    call += 1
            # dense-merge lanes into tab_out
            n_m = NETS_PAD * 4 // (128 * 2048)
            for a in range(n_m):
                acc = pool.tile([128, 2048], mybir.dt.bfloat16, tag="macc")
                nc.sync.dma_start(
                    out=acc[:],
                    in_=tables[0][:].rearrange(
                        "(a p f) d -> a p (f d)", p=128, f=512)[a])
                for l in range(1, _LANES):
                    tl = pool.tile([128, 2048], mybir.dt.bfloat16, tag="mtl")
                    nc.sync.dma_start(
                        out=tl[:],
                        in_=tables[l][:].rearrange(
                            "(a p f) d -> a p (f d)", p=128, f=512)[a])
                    nc.vector.tensor_tensor(
                        out=acc[:], in0=acc[:], in1=tl[:],
                        op=mybir.AluOpType.add)
                nc.sync.dma_start(
                    out=tab_out[:].rearrange(
                        "(a p f) d -> a p (f d)", p=128, f=512)[a],
                    in_=acc[:])
    _split_waits(nc)
    return nc


def _build_reduce():
    nc = bass.Bass("TRN2", target_bir_lowering=False, debug=False,
                   num_devices=N_CORES)
    t_in = [
        nc.dram_tensor(f"t{j}", [NETS_PER_CORE, 4], mybir.dt.bfloat16,
                       kind="ExternalInput")
        for j in range(N_CORES)
    ]
    m_in = nc.dram_tensor("mask", [NETS_PER_CORE], mybir.dt.uint8,
                          kind="ExternalInput")
    p_out = nc.dram_tensor("partial", [128, 1], mybir.dt.float32,
                           kind="ExternalOutput")
    NB = 4
    FB = _PB_COLS // NB          # nets per partition per block
    with tile.TileContext(nc) as tc:
        with tc.tile_pool(name="sb", bufs=2) as pool, \
             tc.tile_pool(name="ac", bufs=1) as apool:
            tot = apool.tile([128, 1], mybir.dt.float32)
            nc.vector.memset(tot[:], 0.0)
            for b in range(NB):
                s = pool.tile([128, FB * 4], mybir.dt.float32, tag="s")
                view = lambda j: t_in[j][:].rearrange(
                    "(p nb f) d -> p nb (f d)", p=128, nb=NB)[:, b]
                s0 = pool.tile([128, FB * 4], mybir.dt.bfloat16, tag="s0")
                nc.sync.dma_start(out=s0[:], in_=view(0))
                nc.vector.tensor_copy(s[:], s0[:])
                for j in range(1, N_CORES):
                    tj = pool.tile([128, FB * 4], mybir.dt.bfloat16, tag="tj")
                    nc.sync.dma_start(out=tj[:], in_=view(j))
                    nc.vector.tensor_tensor(
                        out=s[:], in0=s[:], in1=tj[:], op=mybir.AluOpType.add)
                pos = pool.tile([128, FB * 4], mybir.dt.float32, tag="pos")
                nc.vector.tensor_scalar(
                    pos[:], s[:], 0.0, None, op0=mybir.AluOpType.is_gt)
                nc.vector.tensor_scalar_add(s[:], s[:], 1e-30)
                ln = pool.tile([128, FB * 4], mybir.dt.float32, tag="ln")
                nc.scalar.activation(
                    ln[:], s[:], mybir.ActivationFunctionType.Ln)
                nc.vector.tensor_tensor(
                    out=ln[:], in0=ln[:], in1=pos[:], op=mybir.AluOpType.mult)
                wl = pool.tile([128, FB], mybir.dt.float32, tag="wl")
                nc.vector.tensor_reduce(
                    out=wl[:], in_=ln[:].rearrange("p (f d) -> p f d", d=4),
                    axis=mybir.AxisListType.X, op=mybir.AluOpType.add)
                mu8 = pool.tile([128, FB], mybir.dt.uint8, tag="mu8")
                nc.sync.dma_start(
                    out=mu8[:],
                    in_=m_in[:].rearrange("(p nb f) -> p nb f", p=128, nb=NB)[:, b])
                mf = pool.tile([128, FB], mybir.dt.float32, tag="mf")
                nc.vector.tensor_scalar(
                    mf[:], mu8[:], 0, None, op0=mybir.AluOpType.is_gt)
                nc.vector.tensor_tensor(
                    out=wl[:], in0=wl[:], in1=mf[:], op=mybir.AluOpType.mult)
                red = pool.tile([128, 1], mybir.dt.float32, tag="red")
                nc.vector.tensor_reduce(
                    out=red[:], in_=wl[:], axis=mybir.AxisListType.X,
                    op=mybir.AluOpType.add)
                nc.vector.tensor_tensor(
                    out=tot[:], in0=tot[:], in1=red[:], op=mybir.AluOpType.add)
            nc.sync.dma_start(out=p_out[:], in_=tot[:])
    _split_waits(nc)
    return nc


def _get(name, builder):
    if name not in _nc_cache:
        _nc_cache[name] = builder()
    return _nc_cache[name]


def kernel(pos, pin2net_map, net_mask):
    pos = np.asarray(pos, dtype=np.float32)
    pin2net_map = np.asarray(pin2net_map, dtype=np.int32)
    net_mask = np.asarray(net_mask)

    x = pos[:NUM_PINS]
    y = pos[NUM_PINS:]

    nc_s = _get("s", _build_scatter)
    in_maps = []
    for i in range(N_CORES):
        sl = slice(i * PINS_PER_CORE, (i + 1) * PINS_PER_CORE)
        in_maps.append({
            "x": np.ascontiguousarray(x[sl]).astype(np.float16),
            "y": np.ascontiguousarray(y[sl]).astype(np.float16),
            "net": np.ascontiguousarray(pin2net_map[sl]),
        })
    t0 = time.time()
    res_s = run_bass_kernel_spmd(nc_s, in_maps, list(range(N_CORES)))
    LAUNCH_WALLS["scatter"] = time.time() - t0
    tabs = [res_s.results[i]["tab"] for i in range(N_CORES)]

    maskp = np.zeros(NETS_PAD, dtype=np.uint8)
    maskp[:NUM_NETS] = net_mask.astype(np.uint8)

    nc_r = _get("r", _build_reduce)
    in_maps_r = []
    for i in range(N_CORES):
        sl = slice(i * NETS_PER_CORE, (i + 1) * NETS_PER_CORE)
        m = {f"t{j}": np.ascontiguousarray(tabs[j][sl]) for j in range(N_CORES)}
        m["mask"] = np.ascontiguousarray(maskp[sl])
        in_maps_r.append(m)
    t0 = time.time()
    res_r = run_bass_kernel_spmd(nc_r, in_maps_r, list(range(N_CORES)))
    LAUNCH_WALLS["reduce"] = time.time() - t0
    total = 0.0
    for i in range(N_CORES):
        total += float(res_r.results[i]["partial"].sum())
    return np.float32(GAMMA * total)
